# revision 2
# baseline (speedup 1.0000x reference)
"""Trainium2 Bass kernel for cross-attention (b=4, nq=2048, nkv=1024,
qdim=1024, cdim=768, heads=16, dim_head=64).

Sharding: 8 cores = batch(4) x nq-half(2). Each core computes a disjoint
[1024, 1024] slice of the output; no collectives.

Host-side prep (inside kernel(), per call):
  - kv compaction: only the ~50% unmasked context rows are shipped,
    padded to a multiple of 128 (capacity NKVC, 640 for the reference
    mask). Pad rows get exp-bias -30 so they vanish from both the
    attention numerator and denominator. Exact w.r.t. the reference
    masked softmax.
  - x and context are pre-transposed and cast to bf16 on the host, so
    the kernel needs no PE transposes.
  - Weights cast to bf16; bo pre-broadcast to [128, 1024] fp32.

Per-core algorithm (all matmuls bf16 in, fp32 PSUM out):
  V  = ctx_c @ Wv            [nkv, inner]
  KT = Wk^T @ ctx_c^T        [inner, nkv]   (chunk p = head pair 2p,2p+1)
  QT = Wq^T @ x^T            [inner, nq]
  per (head-pair p, q-half hf):
    for kv-chunk c:
      S^T = K_h @ Q_h^T      (2 heads row-tiled K=64, N=512 each)
      ES  = exp(SCALE*S^T + maskbias[c])   (ScalarE, PSUM->SBUF bf16)
      PO += V_h^T @ ES       (2 heads col-tiled M=64)
      PR += ones^T @ ES      (rowsum, col-tiled M=64)
    OT = PO * (1/PR)         (DVE reciprocal + mul -> bf16)
  out = OT^T @ Wo + bo       (bias added during PSUM evacuation)
"""

import numpy as np
from contextlib import ExitStack

import concourse.bass as bass
import concourse.mybir as mybir
import concourse.tile as tile
from concourse import bacc
from concourse.bass_utils import run_bass_kernel_spmd

F32 = mybir.dt.float32
BF16 = mybir.dt.bfloat16
AF = mybir.ActivationFunctionType
NP_BF16 = mybir.dt.np(BF16)

NQ = 1024      # queries per core
QD = 1024
CD = 768
H = 16
D = 64
INNER = 1024
SCALE = D ** -0.5
P = 128
NQC = NQ // P      # 8
QDC = QD // P      # 8
CDC = CD // P      # 6
HP = H // 2        # 8 head pairs
PAD_BIAS = -30.0   # exp(-30) ~ 1e-13: kills padded kv rows


def _emit(tc, io, nkc):
    nc = tc.nc
    xt_d, ct_d, bias_d, wq_d, wk_d, wv_d, wo_d, bob_d, out_d = io
    nkv = nkc * P

    with ExitStack() as top:
        const = top.enter_context(tc.tile_pool(name="const", bufs=1))
        ones64 = const.tile([P, D], BF16, tag="ones64")
        nc.vector.memset(ones64[:], 1.0)
        bias2 = const.tile([P, nkc], F32, tag="bias2")
        nc.sync.dma_start(out=bias2[:], in_=bias_d)
        bob = const.tile([P, QD], F32, tag="bob")
        nc.sync.dma_start(out=bob[:], in_=bob_d)

        big = top.enter_context(tc.tile_pool(name="big", bufs=1))
        xt = big.tile([P, QDC * NQ], BF16, tag="xt")    # x^T: chunk k at k*NQ
        ct = big.tile([P, CDC * nkv], BF16, tag="ct")   # ctx^T: chunk k at k*nkv
        qt = big.tile([P, HP * NQ], BF16, tag="qt")     # Q^T: chunk p at p*NQ
        kt = big.tile([P, HP * nkv], BF16, tag="kt")    # K^T: chunk p at p*nkv
        vt = big.tile([P, nkc * INNER], BF16, tag="vt")  # V: kv-chunk c at c*INNER
        ot = big.tile([P, HP * NQ], BF16, tag="ot")     # O^T: chunk p at p*NQ

        for k in range(CDC):
            nc.sync.dma_start(out=ct[:, k * nkv:(k + 1) * nkv],
                              in_=ct_d[k * P:(k + 1) * P, :])

        # ------------- V = ctx_c @ Wv   [nkv, inner] -------------
        with tc.tile_pool(name="wv", bufs=CDC) as wv_pool, \
             tc.tile_pool(name="pj_ps", bufs=4, space="PSUM") as pj_ps:
            wv = [wv_pool.tile([P, INNER], BF16, tag="wv", name=f"wv{k}")
                  for k in range(CDC)]
            for k in range(CDC):
                nc.sync.dma_start(out=wv[k][:], in_=wv_d[k * P:(k + 1) * P, :])
            for c in range(nkc):
                for nh in range(2):
                    ps = pj_ps.tile([P, 512], F32, tag="pj", name=f"pjv{c}_{nh}")
                    for k in range(CDC):
                        nc.tensor.matmul(
                            ps[:], ct[:, k * nkv + c * P: k * nkv + (c + 1) * P],
                            wv[k][:, nh * 512:(nh + 1) * 512],
                            start=(k == 0), stop=(k == CDC - 1))
                    nc.vector.tensor_copy(
                        vt[:, c * INNER + nh * 512: c * INNER + (nh + 1) * 512],
                        ps[:])

        # ------------- KT = Wk^T @ ctx_c^T   [inner, nkv] -------------
        with tc.tile_pool(name="wk", bufs=CDC) as wk_pool, \
             tc.tile_pool(name="pj_ps2", bufs=4, space="PSUM") as pj_ps2:
            wk = [wk_pool.tile([P, INNER], BF16, tag="wk", name=f"wk{k}")
                  for k in range(CDC)]
            for k in range(CDC):
                nc.sync.dma_start(out=wk[k][:], in_=wk_d[k * P:(k + 1) * P, :])
            nparts = [(i * 512, min(512, nkv - i * 512))
                      for i in range((nkv + 511) // 512)]
            for p in range(HP):
                for off, nn in nparts:
                    ps = pj_ps2.tile([P, nn], F32, tag="pjk",
                                     name=f"pjk{p}_{off}")
                    for k in range(CDC):
                        nc.tensor.matmul(
                            ps[:], wk[k][:, p * P:(p + 1) * P],
                            ct[:, k * nkv + off: k * nkv + off + nn],
                            start=(k == 0), stop=(k == CDC - 1))
                    nc.vector.tensor_copy(
                        kt[:, p * nkv + off: p * nkv + off + nn], ps[:])

        # ------------- QT = Wq^T @ x^T   [inner, nq] -------------
        with tc.tile_pool(name="wq", bufs=QDC) as wq_pool, \
             tc.tile_pool(name="pj_ps3", bufs=4, space="PSUM") as pj_ps3:
            wq = [wq_pool.tile([P, INNER], BF16, tag="wq", name=f"wq{k}")
                  for k in range(QDC)]
            for k in range(QDC):
                nc.sync.dma_start(out=xt[:, k * NQ:(k + 1) * NQ],
                                  in_=xt_d[k * P:(k + 1) * P, :])
                nc.sync.dma_start(out=wq[k][:], in_=wq_d[k * P:(k + 1) * P, :])
            for p in range(HP):
                for hf in range(2):
                    ps = pj_ps3.tile([P, 512], F32, tag="pjq",
                                     name=f"pjq{p}_{hf}")
                    for k in range(QDC):
                        nc.tensor.matmul(
                            ps[:], wq[k][:, p * P:(p + 1) * P],
                            xt[:, k * NQ + hf * 512: k * NQ + (hf + 1) * 512],
                            start=(k == 0), stop=(k == QDC - 1))
                    nc.vector.tensor_copy(
                        qt[:, p * NQ + hf * 512: p * NQ + (hf + 1) * 512],
                        ps[:])

        # ------------- attention -------------
        with tc.tile_pool(name="es", bufs=4) as es_pool, \
             tc.tile_pool(name="rt", bufs=2) as rt_pool, \
             tc.tile_pool(name="ps_s", bufs=2, space="PSUM") as ps_s, \
             tc.tile_pool(name="ps_o", bufs=2, space="PSUM") as ps_o, \
             tc.tile_pool(name="ps_r", bufs=2, space="PSUM") as ps_r:
            for p in range(HP):
                po = [ps_o.tile([P, 512], F32, tag="po", name=f"po{p}_{hf}")
                      for hf in range(2)]
                pr = [ps_r.tile([P, 512], F32, tag="pr", name=f"pr{p}_{hf}")
                      for hf in range(2)]
                # Two q-half streams interleaved, with PV/rowsum emitted one
                # kv-chunk behind S/exp: PE never waits on a fresh exp.
                es_by = {}
                for c in range(nkc + 1):
                    if c < nkc:
                        for hf in range(2):
                            ps = ps_s.tile([P, NQ], F32, tag="ss",
                                           name=f"ss{p}_{hf}_{c}")
                            for hh in range(2):
                                nc.tensor.matmul(
                                    ps[:, hh * 512:(hh + 1) * 512],
                                    kt[hh * D:(hh + 1) * D,
                                       p * nkv + c * P: p * nkv + (c + 1) * P],
                                    qt[hh * D:(hh + 1) * D,
                                       p * NQ + hf * 512:
                                       p * NQ + (hf + 1) * 512],
                                    start=True, stop=True,
                                    tile_position=(hh * D, 0))
                            es = es_pool.tile([P, NQ], BF16, tag="es",
                                              name=f"es{p}_{hf}_{c}")
                            nc.scalar.activation(es[:], ps[:], AF.Exp,
                                                 scale=float(SCALE),
                                                 bias=bias2[:, c:c + 1])
                            es_by[(c, hf)] = es
                    if c >= 1:
                        cc = c - 1
                        for hf in range(2):
                            es = es_by.pop((cc, hf))
                            for hh in range(2):
                                h = 2 * p + hh
                                esl = es[:, hh * 512:(hh + 1) * 512]
                                nc.tensor.matmul(
                                    po[hf][hh * D:(hh + 1) * D, :],
                                    vt[:, cc * INNER + h * D:
                                       cc * INNER + (h + 1) * D],
                                    esl,
                                    start=(cc == 0), stop=(cc == nkc - 1),
                                    tile_position=(0, hh * D),
                                    skip_group_check=True)
                                nc.tensor.matmul(
                                    pr[hf][hh * D:(hh + 1) * D, :],
                                    ones64[:], esl,
                                    start=(cc == 0), stop=(cc == nkc - 1),
                                    tile_position=(0, hh * D),
                                    skip_group_check=True)
                for hf in range(2):
                    rt = rt_pool.tile([P, 512], F32, tag="rt",
                                      name=f"rt{p}_{hf}")
                    with nc.allow_low_precision(reason="softmax reciprocal"):
                        nc.vector.reciprocal(rt[:], pr[hf][:])
                    nc.vector.tensor_mul(
                        ot[:, p * NQ + hf * 512: p * NQ + (hf + 1) * 512],
                        po[hf][:], rt[:])

        # ------------- out = OT^T @ Wo + bo -------------
        with tc.tile_pool(name="wo", bufs=QDC) as wo_pool, \
             tc.tile_pool(name="out_ps", bufs=4, space="PSUM") as out_ps, \
             tc.tile_pool(name="out_sb", bufs=3) as out_sb:
            wo = [wo_pool.tile([P, QD], BF16, tag="wo", name=f"wo{k}")
                  for k in range(QDC)]
            for k in range(QDC):
                nc.sync.dma_start(out=wo[k][:], in_=wo_d[k * P:(k + 1) * P, :])
            for m in range(NQC):
                for n in range(2):
                    ps = out_ps.tile([P, 512], F32, tag="ops",
                                     name=f"ops{m}_{n}")
                    for k in range(QDC):
                        nc.tensor.matmul(
                            ps[:],
                            ot[:, k * NQ + m * P: k * NQ + (m + 1) * P],
                            wo[k][:, n * 512:(n + 1) * 512],
                            start=(k == 0), stop=(k == QDC - 1))
                    sb = out_sb.tile([P, 512], F32, tag="osb",
                                     name=f"osb{m}_{n}")
                    nc.vector.tensor_add(sb[:], ps[:],
                                         bob[:, n * 512:(n + 1) * 512])
                    nc.sync.dma_start(
                        out=out_d[m * P:(m + 1) * P, n * 512:(n + 1) * 512],
                        in_=sb[:])


_CACHED = {}


def _build(iters=1, nkc=5):
    key = (iters, nkc)
    if key in _CACHED:
        return _CACHED[key]
    nkv = nkc * P
    nc = bacc.Bacc("TRN2", debug=False, target_bir_lowering=False)
    xt_d = nc.dram_tensor("xt", [QD, NQ], BF16, kind="ExternalInput").ap()
    ct_d = nc.dram_tensor("ct", [CD, nkv], BF16, kind="ExternalInput").ap()
    bias_d = nc.dram_tensor("biasm", [P, nkc], F32, kind="ExternalInput").ap()
    wq_d = nc.dram_tensor("wq", [QD, INNER], BF16, kind="ExternalInput").ap()
    wk_d = nc.dram_tensor("wk", [CD, INNER], BF16, kind="ExternalInput").ap()
    wv_d = nc.dram_tensor("wv", [CD, INNER], BF16, kind="ExternalInput").ap()
    wo_d = nc.dram_tensor("wo", [INNER, QD], BF16, kind="ExternalInput").ap()
    bob_d = nc.dram_tensor("bob", [P, QD], F32, kind="ExternalInput").ap()
    out_d = nc.dram_tensor("out", [NQ, QD], F32, kind="ExternalOutput").ap()
    io = (xt_d, ct_d, bias_d, wq_d, wk_d, wv_d, wo_d, bob_d, out_d)
    with tile.TileContext(nc) as tc:
        for _ in range(iters):
            _emit(tc, io, nkc)
    nc.compile()
    _CACHED[key] = nc
    return nc


def _capacity(mask):
    """kv capacity: max unmasked count over batches, rounded up to 128."""
    counts = np.asarray(mask).astype(np.int64).sum(axis=1)
    cap = int(max(int(counts.max()), 1))
    return min((cap + P - 1) // P, 8)


def make_in_maps(x, context, mask, Wq, Wk, Wv, Wo, bo, nkc=None):
    x = np.asarray(x, dtype=np.float32)
    context = np.asarray(context, dtype=np.float32)
    mask = np.asarray(mask).astype(bool)
    if nkc is None:
        nkc = _capacity(mask)
    nkv = nkc * P
    wq = np.ascontiguousarray(np.asarray(Wq, dtype=np.float32)).astype(NP_BF16)
    wk = np.ascontiguousarray(np.asarray(Wk, dtype=np.float32)).astype(NP_BF16)
    wv = np.ascontiguousarray(np.asarray(Wv, dtype=np.float32)).astype(NP_BF16)
    wo = np.ascontiguousarray(np.asarray(Wo, dtype=np.float32)).astype(NP_BF16)
    bob = np.ascontiguousarray(
        np.broadcast_to(np.asarray(bo, dtype=np.float32), (P, QD)))
    in_maps = []
    for b in range(4):
        idx = np.nonzero(mask[b])[0]
        n = len(idx)
        ctx_c = np.zeros((nkv, CD), dtype=np.float32)
        ctx_c[:n] = context[b][idx]
        ct_t = np.ascontiguousarray(ctx_c.T).astype(NP_BF16)
        biasv = np.full(nkv, PAD_BIAS, dtype=np.float32)
        biasv[:n] = 0.0
        bias2 = np.ascontiguousarray(biasv.reshape(nkc, P).T)
        for qh in range(2):
            xt_t = np.ascontiguousarray(
                x[b, qh * NQ:(qh + 1) * NQ, :].T).astype(NP_BF16)
            in_maps.append({
                "xt": xt_t, "ct": ct_t, "biasm": bias2,
                "wq": wq, "wk": wk, "wv": wv, "wo": wo, "bob": bob,
            })
    return in_maps


def run_sharded(x, context, mask, Wq, Wk, Wv, Wo, bo, trace=False, **kw):
    nkc = _capacity(np.asarray(mask).astype(bool))
    nc = _build(1, nkc)
    in_maps = make_in_maps(x, context, mask, Wq, Wk, Wv, Wo, bo, nkc=nkc)
    res = run_bass_kernel_spmd(nc, in_maps, list(range(8)), trace=trace, **kw)
    out = np.empty((4, 2 * NQ, QD), dtype=np.float32)
    for i in range(8):
        b, qh = divmod(i, 2)
        out[b, qh * NQ:(qh + 1) * NQ, :] = res.results[i]["out"]
    return out, res


def kernel(x, context, mask, Wq, Wk, Wv, Wo, bo):
    out, _ = run_sharded(x, context, mask, Wq, Wk, Wv, Wo, bo, trace=False)
    return out
